# revision 39
# baseline (speedup 1.0000x reference)
"""Trainium2 Bass kernel for the AggregateLayer pooling problem.

reference semantics (per batch b):
    dot_w[j] = <pref[b,j,:], c[b,0,:]>                      (j = 0..63)
    t_w[j]   = 1 / |t_pref[b,0,j] - t_c[b,0]|
    w        = softmax(dot_w + t_w)                          (over j)
    u[b,0,:] = sum_j w[j] * pref[b,j,:]

Strategy: pure data parallel over 8 NeuronCores (1024 batches each).

HYBRID split per core: the first PEG*128 batches run a TensorEngine-heavy
pipeline (PE transposes + 2-column dot matmuls + PE weighted sum, pair
tiles via interleaved cast-DMA); the remaining batches run a Vector-heavy
pipeline (contiguous fp32 loads + Scalar cast, DVE elementwise dots,
PE only for the pair-compress weighted sum). The two pipelines use
disjoint heavy engines and separate DMA rings, so they overlap.
"""

import numpy as np
from contextlib import ExitStack

import concourse.bass as bass
import concourse.tile as tile
from concourse import mybir
from concourse.masks import make_identity
from concourse.bass_utils import run_bass_kernel_spmd
import concourse.bass2jax as _b2j


def _split_multiwait(bir: dict) -> int:
    """Walrus in this container rejects >1 sync-wait per instruction.

    Hoist excess waits onto NoOps inserted just before the instruction on
    the same engine (program order within the engine stream preserves the
    wait semantics exactly).
    """
    n = 0
    for fn in bir["functions"]:
        for blk in fn["blocks"]:
            out = []
            for inst in blk["instructions"]:
                si = inst.get("sync_info")
                waits = si.get("on_wait") if si else None
                if waits and len(waits) > 1:
                    for w in waits[:-1]:
                        out.append(
                            {
                                "opcode": "NoOp",
                                "engine": inst["engine"],
                                "name": f"{inst['name']}-xw{n}",
                                "ins": [],
                                "outs": [],
                                "sync_info": {"on_update": [], "on_wait": [w]},
                            }
                        )
                        n += 1
                    si["on_wait"] = [waits[-1]]
                out.append(inst)
            blk["instructions"] = out
    return n


_orig_compile_bir_kernel = _b2j.compile_bir_kernel


def _legalizing_compile_bir_kernel(ant_bir_str, *args, **kwargs):
    import orjson

    bir = orjson.loads(ant_bir_str)
    _split_multiwait(bir)
    return _orig_compile_bir_kernel(orjson.dumps(bir), *args, **kwargs)


_b2j.compile_bir_kernel = _legalizing_compile_bir_kernel

F32 = mybir.dt.float32
F16 = mybir.dt.float16
Alu = mybir.AluOpType
Act = mybir.ActivationFunctionType
Axis = mybir.AxisListType

B, N, D = 8192, 64, 128
NCORES = 8
BPC = B // NCORES          # 1024 batches per core

# --- PE-heavy side: groups of 128 batches (pair tiles) -------------------
GROUP = 128
NTILES = GROUP // 2        # 64 two-batch tiles per group
NPAIR = GROUP // 2         # batch-pairs per group (softmax partitions)
PEG = 3                    # number of PE groups
BOFF = PEG * GROUP         # batches handled by the PE side

# --- Vector-heavy side: chunks of 64 batches (contiguous loads) ----------
CHUNK = 64
NCHUNK = (BPC - BOFF) // CHUNK
NT = 32                    # j%32 positions per partition row-block
ROWS = CHUNK * N           # 4096 flat rows per chunk


# ======================= PE-heavy group pipeline =========================

def _build_group(nc, gpools, gconsts, gaps, g):
    (pref_rows, u_all, c32a, gtpa, gtca) = gaps
    ident16, ident32 = gconsts
    (g_p16, g_pt, g_small, g_ps_pt, g_ps_mm, g_ps_small) = gpools

    r0 = g * GROUP * N
    b0 = g * GROUP

    # ---- c transpose + t_w ----------------------------------------------
    cg16 = g_small.tile([GROUP, D], F16, tag="cg16")
    nc.vector.tensor_copy(out=cg16[:], in_=c32a[:, g, :])
    ct_ps = g_ps_small.tile([128, GROUP], F16, tag="sm_ps")
    nc.tensor.transpose(
        out=ct_ps[:], in_=cg16[:], identity=ident16[0:GROUP, 0:GROUP]
    )
    ct16 = g_small.tile([128, GROUP], F16, tag="ct16")
    nc.vector.tensor_copy(out=ct16[:], in_=ct_ps[:])

    tw = g_small.tile([NPAIR, 2, N], F32, tag="tw")
    for s in range(2):
        nc.vector.tensor_scalar_sub(
            out=tw[:, s, :], in0=gtpa[:, g, s, :], scalar1=gtca[:, g, s : s + 1]
        )
    nc.scalar.activation(out=tw[:], in_=tw[:], func=Act.Abs)
    nc.vector.reciprocal(out=tw[:], in_=tw[:])

    # ---- pref load with cast-in-DMA (SWDGE), in 16-tile chunks -----------
    HT = 16
    p16 = g_p16.tile([128, NTILES, D], F16, tag="p16")
    for h0 in range(0, NTILES, HT):
        rh = r0 + h0 * 128
        nc.gpsimd.dma_start(
            out=p16[:, h0 : h0 + HT, :],
            in_=pref_rows[rh : rh + HT * 128, :].rearrange(
                "(t p) d -> p t d", p=128
            ),
        )

    # ---- transposes + dot matmuls ---------------------------------------
    pts = g_pt.tile([128, NTILES, 128], F16, tag="pts")
    CH = 8
    for t0 in range(0, NTILES, CH):
        pt_ps = g_ps_pt.tile([128, CH, 128], F16, tag="pt_ps")
        for k in range(CH):
            nc.tensor.transpose(
                out=pt_ps[:, k, :], in_=p16[:, t0 + k, :], identity=ident16[:]
            )
        if (t0 // CH) % 3 == 2:
            nc.vector.tensor_copy(out=pts[:, t0 : t0 + CH, :], in_=pt_ps[:])
        else:
            nc.scalar.copy(out=pts[:, t0 : t0 + CH, :], in_=pt_ps[:])

    ps_dots = g_ps_mm.tile([128, NTILES, 2], F32, tag="mm_ps")
    for t in range(NTILES):
        nc.tensor.matmul(
            out=ps_dots[:, t, :],
            lhsT=pts[:, t, :],
            rhs=ct16[:, 2 * t : 2 * t + 2],
            start=(t == 0),
            stop=(t == NTILES - 1),
        )

    # valid dots sit at [row, parity=row//64]: extract the two halves
    dotw_rows = g_small.tile([128, NTILES], F32, tag="dotw_rows")
    nc.scalar.copy(out=dotw_rows[0:64, :], in_=ps_dots[0:64, :, 0])
    nc.scalar.copy(out=dotw_rows[64:128, :], in_=ps_dots[64:128, :, 1])

    # transpose [128(row), nt] -> [nt, 128(row)] => pair-major dots
    dr_ps = g_ps_small.tile([NPAIR, 128], F32, tag="sm_ps")
    nc.tensor.transpose(out=dr_ps[:], in_=dotw_rows[:], identity=ident32[:])

    # ---- softmax over j (segmented, pair-major) --------------------------
    w = g_small.tile([NPAIR, 2, N], F32, tag="w")
    nc.vector.tensor_add(
        out=w[:],
        in0=dr_ps[:].rearrange("t (two n) -> t two n", two=2),
        in1=tw[:],
    )
    nmx = g_small.tile([NPAIR, 2], F32, tag="nmx")
    nc.vector.tensor_reduce(
        out=nmx[:], in_=w[:], axis=Axis.X, op=Alu.max, negate=True
    )
    e = g_small.tile([NPAIR, 2, N], F32, tag="e")
    for s in range(2):
        nc.scalar.activation(
            out=e[:, s, :],
            in_=w[:, s, :],
            func=Act.Exp,
            bias=nmx[:, s : s + 1],
            scale=1.0,
        )
    z = g_small.tile([NPAIR, 2], F32, tag="z")
    nc.vector.reduce_sum(out=z[:], in_=e[:], axis=Axis.X)
    rz = g_small.tile([NPAIR, 2], F32, tag="rz")
    nc.vector.reciprocal(out=rz[:], in_=z[:])
    wn16 = g_small.tile([NPAIR, 2, N], F16, tag="wn16")
    for s in range(2):
        nc.vector.tensor_scalar_mul(
            out=wn16[:, s, :], in0=e[:, s, :], scalar1=rz[:, s : s + 1]
        )

    # ---- build W_MAT [row, t, parity] (block structure, zeros elsewhere) --
    wc_ps = g_ps_small.tile([128, NTILES], F16, tag="sm_ps")
    nc.tensor.transpose(
        out=wc_ps[:],
        in_=wn16[:].rearrange("t two n -> t (two n)"),
        identity=ident16[0:NPAIR, 0:NPAIR],
    )
    wcol16 = g_small.tile([128, NTILES], F16, tag="wcol16")
    nc.vector.tensor_copy(out=wcol16[:], in_=wc_ps[:])
    wmat16 = g_small.tile([128, NTILES, 2], F16, tag="wmat16")
    nc.vector.memset(wmat16[:], 0.0)
    nc.vector.tensor_copy(out=wmat16[0:64, :, 0], in_=wcol16[0:64, :])
    nc.vector.tensor_copy(out=wmat16[64:128, :, 1], in_=wcol16[64:128, :])

    # ---- weighted-sum matmuls + store, in half-groups --------------------
    HB = NTILES // 2
    for h in range(2):
        ps_ut = g_ps_mm.tile([128, HB, 2], F32, tag="mm_ps")
        for k in range(HB):
            t = h * HB + k
            nc.tensor.matmul(
                out=ps_ut[:, k, :],
                lhsT=p16[:, t, :],
                rhs=wmat16[:, t, :],
                start=(k == 0),
                stop=(k == HB - 1),
            )
        uts = g_small.tile([128, GROUP // 2], F32, tag="uts")
        nc.vector.tensor_copy(
            out=uts[:], in_=ps_ut[:].rearrange("d t two -> d (t two)")
        )
        ug_ps = g_ps_small.tile([GROUP // 2, 128], F32, tag="sm_ps")
        nc.tensor.transpose(out=ug_ps[:], in_=uts[:], identity=ident32[:])
        ug = g_small.tile([GROUP // 2, 128], F32, tag="ug")
        nc.vector.tensor_copy(out=ug[:], in_=ug_ps[:])
        bh = b0 + h * (GROUP // 2)
        nc.gpsimd.dma_start(
            out=u_all[bh : bh + GROUP // 2, :].rearrange(
                "b (x d) -> b x d", x=2
            ),
            in_=ug[:].rearrange("b (x d) -> b x d", x=2),
        )


# ====================== Vector-heavy chunk pipeline ======================

def _stage_load(nc, pools, aps, tiles, k):
    """Issue chunk k's DMAs + fp32->fp16 cast (runs ahead of compute)."""
    (p_pref32, p_pref, p_y, p_w, p_cexp, p_sm, p_u, ps_cexp, ps_u, ps_z) = pools
    (pref_rows, tp_rows, u_rows) = aps
    r0 = (BOFF + k * CHUNK) * N

    p32 = p_pref32.tile([128, NT, D], F32, tag="p32")
    nc.sync.dma_start(
        out=p32[:],
        in_=pref_rows[r0 : r0 + ROWS, :].rearrange("(p t) d -> p t d", p=128),
    )
    p16 = p_pref.tile([128, NT, D], F16, tag="p16")
    nc.scalar.copy(out=p16[:], in_=p32[:])

    tp_k = p_sm.tile([128, NT], F32, tag="tpk")
    b0 = BOFF + k * CHUNK
    nc.sync.dma_start(
        out=tp_k[:],
        in_=tp_rows[b0 : b0 + CHUNK, :].rearrange("q (h t) -> (q h) t", h=2),
    )
    tiles[k] = (p16, tp_k)


def _phase_a(nc, pools, consts, tiles, k):
    """cexp matmul + Y-mult."""
    (p_pref32, p_pref, p_y, p_w, p_cexp, p_sm, p_u, ps_cexp, ps_u, ps_z) = pools
    (sp16, spT16, cg16, tca) = consts
    (p16, tp_k) = tiles[k]

    cexp_ps = ps_cexp.tile([128, D], F32, tag="cexp_ps")
    nc.tensor.matmul(
        out=cexp_ps[:], lhsT=spT16[:], rhs=cg16[:, k, :], start=True, stop=True
    )
    cexp16 = p_cexp.tile([128, D], F16, tag="cexp16")
    nc.scalar.copy(out=cexp16[:], in_=cexp_ps[:])

    y16 = p_y.tile([128, NT, D], F16, tag="y16")
    nc.vector.tensor_tensor(
        out=y16[:],
        in0=p16[:],
        in1=cexp16[:].unsqueeze(1).broadcast_to((128, NT, D)),
        op=Alu.mult,
    )
    tiles[k] = (p16, tp_k, y16)


def _phase_b(nc, pools, consts, tiles, k):
    """reduce + t_w + -max + pair-merge DMA issue."""
    (p_pref32, p_pref, p_y, p_w, p_cexp, p_sm, p_u, ps_cexp, ps_u, ps_z) = pools
    (sp16, spT16, cg16, tca) = consts
    (p16, tp_k, y16) = tiles[k]

    dots = p_sm.tile([128, NT], F32, tag="dots")
    nc.vector.reduce_sum(out=dots[:], in_=y16[:], axis=Axis.X)

    # tca holds -t_c, so Abs(t_pref + bias) fuses the subtract
    adtw = p_sm.tile([128, NT], F32, tag="adtw")
    nc.scalar.activation(
        out=adtw[:], in_=tp_k[:], func=Act.Abs, bias=tca[:, k : k + 1]
    )
    tw = p_sm.tile([128, NT], F32, tag="tw")
    nc.vector.reciprocal(out=tw[:], in_=adtw[:])
    wpre = p_sm.tile([128, NT], F32, tag="wpre")
    nc.vector.tensor_add(out=wpre[:], in0=dots[:], in1=tw[:])
    nmh = p_sm.tile([128, 1], F32, tag="nmh")
    nc.vector.tensor_reduce(
        out=nmh[:], in_=wpre[:], axis=Axis.X, op=Alu.max, negate=True
    )
    nms = p_sm.tile([128, 1], F32, tag="nms")
    nmh_v = nmh[:].rearrange("(q h) one -> q h one", h=2)
    nms_v = nms[:].rearrange("(q h) one -> q h one", h=2)
    nc.sync.dma_start(out=nms_v[:, 0, :], in_=nmh_v[:, 1, :])
    nc.sync.dma_start(out=nms_v[:, 1, :], in_=nmh_v[:, 0, :])
    tiles[k] = (p16, wpre, nmh, nms)


def _phase_c(nc, pools, consts, aps, tiles, k):
    """exp + Z + weight build + weighted sum + store."""
    (p_pref32, p_pref, p_y, p_w, p_cexp, p_sm, p_u, ps_cexp, ps_u, ps_z) = pools
    (sp16, spT16, cg16, tca) = consts
    (pref_rows, tp_rows, u_rows) = aps
    (p16, wpre, nmh, nms) = tiles.pop(k)

    nm = p_sm.tile([128, 1], F32, tag="nm")
    nc.vector.tensor_tensor(out=nm[:], in0=nmh[:], in1=nms[:], op=Alu.min)

    e16 = p_sm.tile([128, NT], F16, tag="e16")
    nc.scalar.activation(
        out=e16[:], in_=wpre[:], func=Act.Exp, bias=nm[:], scale=1.0
    )

    zps = ps_z.tile([CHUNK, NT], F32, tag="zps")
    nc.tensor.matmul(
        out=zps[:], lhsT=sp16[:], rhs=e16[:], start=True, stop=True
    )
    zq = p_sm.tile([CHUNK, 1], F32, tag="zq")
    nc.vector.reduce_sum(out=zq[:], in_=zps[:], axis=Axis.X)
    rzq = p_sm.tile([CHUNK, 1], F32, tag="rzq")
    nc.vector.reciprocal(out=rzq[:], in_=zq[:])

    w16 = p_w.tile([128, NT, CHUNK], F16, tag="w16")
    nc.gpsimd.tensor_tensor(
        out=w16[:],
        in0=e16[:].unsqueeze(2).broadcast_to((128, NT, CHUNK)),
        in1=sp16[:].unsqueeze(1).broadcast_to((128, NT, CHUNK)),
        op=Alu.mult,
    )

    ups = ps_u.tile([CHUNK, D], F32, tag="ups")
    for t in range(NT):
        nc.tensor.matmul(
            out=ups[:],
            lhsT=w16[:, t, :],
            rhs=p16[:, t, :],
            start=(t == 0),
            stop=(t == NT - 1),
        )

    usb = p_u.tile([CHUNK, D], F32, tag="usb")
    nc.scalar.mul(out=usb[:], in_=ups[:], mul=rzq[:])
    b0 = BOFF + k * CHUNK
    nc.sync.dma_start(out=u_rows[b0 : b0 + CHUNK, :], in_=usb[:])


def _build_nc():
    nc = bass.Bass()
    pref = nc.declare_dram_parameter("pref", [BPC, N, D], F32, isOutput=False)
    c = nc.declare_dram_parameter("c", [BPC, 1, D], F32, isOutput=False)
    t_pref = nc.declare_dram_parameter("t_pref", [BPC, 1, N], F32, isOutput=False)
    t_c = nc.declare_dram_parameter("t_c", [BPC, 1], F32, isOutput=False)
    spair = nc.declare_dram_parameter("spair", [128, CHUNK], F16, isOutput=False)
    spairT = nc.declare_dram_parameter("spairT", [CHUNK, 128], F16, isOutput=False)
    u = nc.declare_dram_parameter("u", [BPC, 1, D], F32, isOutput=True)

    pref_rows = pref[:].rearrange("b n d -> (b n) d")
    c_rows = c[:].rearrange("b one d -> (b one) d")
    tp_rows = t_pref[:].rearrange("b one n -> (b one) n")
    tc_rows = t_c[:]
    u_rows = u[:].rearrange("b one d -> (b one) d")

    with ExitStack() as ctx:
        tc = ctx.enter_context(tile.TileContext(nc))
        p_const = ctx.enter_context(tc.tile_pool(name="const", bufs=1))

        # ---- shared constants -------------------------------------------
        ident16 = p_const.tile([128, 128], F16)
        ident32 = p_const.tile([128, 128], F32)
        make_identity(nc, ident16[:])
        make_identity(nc, ident32[:])

        sp16 = p_const.tile([128, CHUNK], F16)
        nc.sync.dma_start(out=sp16[:], in_=spair[:])
        spT16 = p_const.tile([CHUNK, 128], F16)
        nc.sync.dma_start(out=spT16[:], in_=spairT[:])

        # ---- PE-side preloads -------------------------------------------
        c32a = p_const.tile([GROUP, PEG, D], F32)
        nc.sync.dma_start(
            out=c32a[:],
            in_=c_rows[0:BOFF, :].rearrange("(g b) d -> b g d", b=GROUP),
        )
        gtpa = p_const.tile([NPAIR, PEG, 2, N], F32)
        nc.sync.dma_start(
            out=gtpa[:],
            in_=tp_rows[0:BOFF, :].rearrange(
                "(g t two) n -> t g two n", t=NPAIR, two=2
            ),
        )
        gtca = p_const.tile([NPAIR, PEG, 2], F32)
        nc.sync.dma_start(
            out=gtca[:],
            in_=tc_rows[0:BOFF, :].rearrange(
                "(g t two) one -> t g (two one)", t=NPAIR, two=2
            ),
        )

        # ---- Vector-side preloads ---------------------------------------
        cg16 = p_const.tile([CHUNK, NCHUNK, D], F16)
        nc.gpsimd.dma_start(
            out=cg16[:],
            in_=c_rows[BOFF:BPC, :].rearrange("(k q) d -> q k d", q=CHUNK),
        )
        tc64 = p_const.tile([CHUNK, NCHUNK], F32)
        nc.sync.dma_start(
            out=tc64[:],
            in_=tc_rows[BOFF:BPC, :].rearrange("(k q) one -> q (k one)", q=CHUNK),
        )
        # expand t_c to partition pairs with two stride-2 DMAs, negate
        tca = p_const.tile([128, NCHUNK], F32)
        tca_v = tca[:].rearrange("(q h) k -> q h k", h=2)
        nc.sync.dma_start(out=tca_v[:, 0, :], in_=tc64[:])
        nc.sync.dma_start(out=tca_v[:, 1, :], in_=tc64[:])
        nc.vector.tensor_scalar_mul(out=tca[:], in0=tca[:], scalar1=-1.0)

        gconsts = (ident16, ident32)
        gaps = (pref_rows, u_rows, c32a, gtpa, gtca)
        consts = (sp16, spT16, cg16, tca)
        aps = (pref_rows, tp_rows, u_rows)

        # ---- pools: PE side ---------------------------------------------
        g_p16 = ctx.enter_context(tc.tile_pool(name="g_p16", bufs=2))
        g_pt = ctx.enter_context(tc.tile_pool(name="g_pt", bufs=2))
        g_small = ctx.enter_context(tc.tile_pool(name="g_small", bufs=2))
        g_ps_pt = ctx.enter_context(
            tc.tile_pool(name="g_ps_pt", bufs=2, space="PSUM")
        )
        g_ps_mm = ctx.enter_context(
            tc.tile_pool(name="g_ps_mm", bufs=2, space="PSUM")
        )
        g_ps_small = ctx.enter_context(
            tc.tile_pool(name="g_ps_small", bufs=1, space="PSUM")
        )
        gpools = (g_p16, g_pt, g_small, g_ps_pt, g_ps_mm, g_ps_small)

        # ---- pools: Vector side -----------------------------------------
        p_pref32 = ctx.enter_context(tc.tile_pool(name="pref32", bufs=2))
        p_pref = ctx.enter_context(tc.tile_pool(name="pref", bufs=6))
        p_y = ctx.enter_context(tc.tile_pool(name="y", bufs=3))
        p_w = ctx.enter_context(tc.tile_pool(name="w", bufs=2))
        p_cexp = ctx.enter_context(tc.tile_pool(name="cexp", bufs=2))
        p_sm = ctx.enter_context(tc.tile_pool(name="sm", bufs=8))
        p_u = ctx.enter_context(tc.tile_pool(name="u", bufs=3))
        ps_cexp = ctx.enter_context(
            tc.tile_pool(name="ps_cexp", bufs=1, space="PSUM")
        )
        ps_u = ctx.enter_context(tc.tile_pool(name="ps_u", bufs=1, space="PSUM"))
        ps_z = ctx.enter_context(tc.tile_pool(name="ps_z", bufs=1, space="PSUM"))
        pools = (p_pref32, p_pref, p_y, p_w, p_cexp, p_sm, p_u, ps_cexp, ps_u, ps_z)

        # ---- interleaved emission ---------------------------------------
        LOOKAHEAD = 2
        tiles = {}
        for k in range(min(LOOKAHEAD, NCHUNK)):
            _stage_load(nc, pools, aps, tiles, k)

        group_before = {0: 0, 3: 1, 6: 2}  # emit group g before C-step i
        for i in range(NCHUNK + 2):
            if i in group_before:
                _build_group(nc, gpools, gconsts, gaps, group_before[i])
            nxt = i + LOOKAHEAD
            if nxt < NCHUNK:
                _stage_load(nc, pools, aps, tiles, nxt)
            if i < NCHUNK:
                _phase_a(nc, pools, consts, tiles, i)
            if 0 <= i - 1 < NCHUNK:
                _phase_b(nc, pools, consts, tiles, i - 1)
            if i - 2 >= 0:
                _phase_c(nc, pools, consts, aps, tiles, i - 2)

    return nc


def _host_consts():
    # SPAIR[p, q] = 1 if q == p//2 else 0   (pair-compress selector)
    sp = np.zeros((128, CHUNK), dtype=np.float16)
    sp[np.arange(128), np.arange(128) // 2] = 1.0
    spT = np.ascontiguousarray(sp.T)
    return sp, spT


_NC_CACHE = None
LAST_RESULT = None


def kernel(pref, c, t_pref, t_c):
    global _NC_CACHE, LAST_RESULT
    if _NC_CACHE is None:
        _NC_CACHE = _build_nc()
    nc = _NC_CACHE

    pref = np.ascontiguousarray(pref, dtype=np.float32)
    c = np.ascontiguousarray(c, dtype=np.float32)
    t_pref = np.ascontiguousarray(t_pref, dtype=np.float32)
    t_c = np.ascontiguousarray(t_c, dtype=np.float32)
    sp, spT = _host_consts()

    in_maps = []
    for i in range(NCORES):
        s = slice(i * BPC, (i + 1) * BPC)
        in_maps.append(
            {
                "pref": pref[s],
                "c": c[s],
                "t_pref": t_pref[s],
                "t_c": t_c[s],
                "spair": sp,
                "spairT": spT,
            }
        )

    res = run_bass_kernel_spmd(nc, in_maps, list(range(NCORES)))
    LAST_RESULT = res
    return np.concatenate([r["u"] for r in res.results], axis=0)
